# revision 19
# baseline (speedup 1.0000x reference)
"""Bass/Tile TP attention kernel for trn2, 8 NeuronCores.

Sequence-parallel attention tuned for a gapless PE stream (~226 ns per
512-wide fp16 matmul incl. hidden FWL weight loads) and for HBM traffic
(the phases are near the compute/memory ridge):

  - weights pre-scaled by scaling_factor and cast to fp16 on host
  - wq/wk columns host-permuted even/odd per 512-block so RoPE pairs sit
    in separate 128-row tiles -> rope is pure element-wise DVE work
  - phases: norm -> [xq AllGather || Q proj (deep wq prefetch)] ->
    merged K/V loop (K frontloaded, V lags 2 chunks; xq read ONCE) ->
    [kT AllGather || V tail] -> [v half-gathers || scores+softmax] ->
    attnV (wo prefetch) -> out proj
  - qT spilled to DRAM between Q and scores to free SBUF for 3-deep xq
    chunk buffering in the merged loop
  - softmax: 1/64 folded into exp scale, ln64 shift keeps unnormalized
    P/attn in fp16 range, 1/sum applied free at out-proj psum eviction
  - v_b kept in chunk order, gathered in 2 halves, k-index remapped in
    attnV so the second half is never on the critical path

core r owns query row blocks {128*(8j+r) : j=0..3} (causal balance).
"""

import numpy as np

S = 4096
E = 4096
NC = 8
RB = S // NC          # 512 rows per core
P = 128
KO = E // P           # 32 k-tiles
NCH = 8               # 512-wide chunks over S or E
CH = 512
HALF = E // 2
EPS = 1e-6
BASE_THETA = 10000.0
LOG64 = float(np.log(64.0))

_BUILT = None


def _build_nc():
    import concourse.bass as bass
    import concourse.mybir as mybir
    import concourse.tile as tile
    from concourse import bacc
    from concourse.masks import make_identity

    dt16 = mybir.dt.float16
    dt32 = mybir.dt.float32
    AX = mybir.AxisListType.X
    mult = mybir.AluOpType.mult
    addop = mybir.AluOpType.add
    subop = mybir.AluOpType.subtract
    maxop = mybir.AluOpType.max
    Copy = mybir.ActivationFunctionType.Copy
    Exp = mybir.ActivationFunctionType.Exp
    Square = mybir.ActivationFunctionType.Square

    nc = bacc.Bacc(
        "TRN2", target_bir_lowering=False, debug=False, num_devices=NC)

    # I/O (weights arrive fp16, pre-scaled, wq/wk col-permuted even/odd)
    x_r = nc.dram_tensor("x_r", [RB, E], dt32, kind="ExternalInput")
    wq = nc.dram_tensor("wq", [E, E], dt16, kind="ExternalInput")
    wk_c = nc.dram_tensor("wk_c", [E, RB], dt16, kind="ExternalInput")
    wv_c = nc.dram_tensor("wv_c", [E, RB], dt16, kind="ExternalInput")
    wo = nc.dram_tensor("wo", [E, E], dt16, kind="ExternalInput")
    cosq_d = nc.dram_tensor("cosq", [HALF, RB], dt16, kind="ExternalInput")
    sinq_d = nc.dram_tensor("sinq", [HALF, RB], dt16, kind="ExternalInput")
    cosk_d = nc.dram_tensor("cosk", [2 * P, S], dt16, kind="ExternalInput")
    sink_d = nc.dram_tensor("sink", [2 * P, S], dt16, kind="ExternalInput")
    amask = nc.dram_tensor("amask", [RB, 2 * CH], dt16, kind="ExternalInput")
    out_r = nc.dram_tensor("out_r", [RB, E], dt16, kind="ExternalOutput")

    # internal DRAM
    qT_d = nc.dram_tensor("qT_d", [E, RB], dt16)  # roped q^T spill
    xqT_b = nc.dram_tensor("xqT_b", [E, RB], dt16)
    xqT_all = nc.dram_tensor("xqT_all", [NC * E, RB], dt16, addr_space="Shared")
    # kT in 4 column-pair buffers so gathers pipeline under the KV loop
    kT_b = [nc.dram_tensor(f"kT_b{p}", [RB, 2 * CH], dt16) for p in range(4)]
    kT_all = [nc.dram_tensor(f"kT_all{p}", [E, 2 * CH], dt16,
                             addr_space="Shared") for p in range(4)]
    v_b0 = nc.dram_tensor("v_b0", [S // 2, RB], dt16)
    v_b1 = nc.dram_tensor("v_b1", [S // 2, RB], dt16)
    v_all0 = nc.dram_tensor("v_all0", [NC * S // 2, RB], dt16, addr_space="Shared")
    v_all1 = nc.dram_tensor("v_all1", [NC * S // 2, RB], dt16, addr_space="Shared")
    warm_b = nc.dram_tensor("warm_b", [P, 8], dt16)
    warm_all = nc.dram_tensor("warm_all", [NC * P, 8], dt16, addr_space="Shared")
    RG = [list(range(NC))]

    with tile.TileContext(nc) as tc:
        with tc.tile_pool(name="const", bufs=1) as constp, \
             tc.tile_pool(name="psQ", bufs=4, space="PSUM") as psQ, \
             tc.tile_pool(name="pstr", bufs=4, space="PSUM") as pstr:

            ident = constp.tile([P, P], dt16, tag="ident")
            make_identity(nc, ident)
            rinv_all = constp.tile([P, 4], dt32, tag="rinv_all")

            # tiny warmup gather: absorbs the first-collective rendezvous
            # barrier (core-start skew) under the norm/Q phases
            warm = constp.tile([P, 8], dt16, tag="warm")
            nc.vector.memset(warm, 0.0)
            nc.sync.dma_start(warm_b[:], warm)
            nc.gpsimd.collective_compute(
                "AllGather", mybir.AluOpType.bypass, replica_groups=RG,
                ins=[warm_b[:]], outs=[warm_all[:]])

            with tc.tile_pool(name="xqTrp", bufs=1) as xqTrp:
                xqT_r = xqTrp.tile([P, KO, RB], dt16, tag="xqT_r")

                # ---- stage A: RMS norm of own rows + transpose ----
                with tc.tile_pool(name="normp", bufs=2) as normp, \
                     tc.tile_pool(name="nstat", bufs=2) as nstat:
                    for t in range(RB // P):
                        x_sb = normp.tile([P, E], dt32, tag="x")
                        nc.sync.dma_start(x_sb, x_r[t * P:(t + 1) * P, :])
                        sq = normp.tile([P, E], dt32, tag="sq")
                        ssum = nstat.tile([P, 1], dt32, tag="ssum")
                        nc.scalar.activation(sq, x_sb, Square, accum_out=ssum)
                        s2 = nstat.tile([P, 1], dt32, tag="s2")
                        nc.vector.tensor_scalar(s2, ssum, 1.0 / E, EPS, mult, addop)
                        s3 = nstat.tile([P, 1], dt32, tag="s3")
                        nc.scalar.sqrt(s3, s2)
                        rinv = nstat.tile([P, 1], dt32, tag="rinv")
                        nc.vector.reciprocal(rinv, s3)
                        xq_sb = normp.tile([P, E], dt16, tag="xq")
                        nc.vector.tensor_scalar_mul(xq_sb, x_sb, rinv[:, 0:1])
                        for c in range(KO):
                            pt = pstr.tile([P, P], dt16, tag="tr")
                            nc.tensor.transpose(pt, xq_sb[:, c * P:(c + 1) * P], ident)
                            nc.scalar.copy(xqT_r[:, c, t * P:(t + 1) * P], pt)
                        nc.sync.dma_start(
                            xqT_b[:, t * P:(t + 1) * P].rearrange(
                                "(ko p) s -> p ko s", p=P),
                            xqT_r[:, :, t * P:(t + 1) * P])

                nc.gpsimd.collective_compute(
                    "AllGather", mybir.AluOpType.bypass, replica_groups=RG,
                    ins=[xqT_b[:]], outs=[xqT_all[:]])

                # ---- stage QT: qT = wq^T @ xq^T for own rows, + rope ----
                # deep wq prefetch (4 groups = 16MB) rides ahead of the
                # gather's HBM traffic; roped q^T spills to DRAM
                with tc.tile_pool(name="wqs", bufs=4) as wqs, \
                     tc.tile_pool(name="qring", bufs=8) as qring, \
                     tc.tile_pool(name="qropes", bufs=2) as qropes, \
                     tc.tile_pool(name="qrtmp", bufs=2) as qrtmp:
                    for mg in range(8):
                        wqt = wqs.tile([P, KO, CH], dt16, tag="wqt")
                        nc.sync.dma_start(
                            wqt, wq[:, mg * CH:(mg + 1) * CH].rearrange(
                                "(ko p) m -> p ko m", p=P))
                        qg = [qring.tile([P, RB], dt16, tag="qg", name=f"qg{i}")
                              for i in range(4)]
                        for m4 in range(4):
                            ps = psQ.tile([P, CH], dt32, tag="mm")
                            for k in range(KO):
                                nc.tensor.matmul(
                                    ps, lhsT=wqt[:, k, m4 * P:(m4 + 1) * P],
                                    rhs=xqT_r[:, k, :],
                                    start=(k == 0), stop=(k == KO - 1))
                            nc.scalar.copy(qg[m4], ps)
                        # rope pairs (h, 2+h) within this 512-col block
                        for h in range(2):
                            j0 = mg * 2 + h  # 128-row block into cosq/sinq
                            cq = qropes.tile([P, RB], dt16, tag="cq")
                            nc.sync.dma_start(cq, cosq_d[j0 * P:(j0 + 1) * P, :])
                            sq_ = qropes.tile([P, RB], dt16, tag="sq")
                            nc.sync.dma_start(sq_, sinq_d[j0 * P:(j0 + 1) * P, :])
                            t1 = qrtmp.tile([P, RB], dt16, tag="t1")
                            nc.vector.tensor_tensor(t1, qg[h], cq, mult)
                            t3 = qrtmp.tile([P, RB], dt16, tag="t3")
                            nc.vector.tensor_tensor(t3, qg[h], sq_, mult)
                            t4 = qrtmp.tile([P, RB], dt16, tag="t4")
                            nc.vector.tensor_tensor(t4, qg[2 + h], sq_, mult)
                            qE = qring.tile([P, RB], dt16, tag="qro", name="qE")
                            nc.vector.tensor_tensor(qE, t1, t4, addop)
                            t5 = qrtmp.tile([P, RB], dt16, tag="t5")
                            nc.vector.tensor_tensor(t5, qg[2 + h], cq, mult)
                            qO = qring.tile([P, RB], dt16, tag="qro", name="qO")
                            nc.vector.tensor_tensor(qO, t5, t3, subop)
                            me, mo = mg * 4 + h, mg * 4 + 2 + h
                            nc.sync.dma_start(qT_d[me * P:(me + 1) * P, :], qE)
                            nc.sync.dma_start(qT_d[mo * P:(mo + 1) * P, :], qO)

            # xqT_r freed
            xqa = xqT_all[:].rearrange("(c ko p) s -> c p ko s", c=NC, p=P)
            with tc.tile_pool(name="wkvp", bufs=1) as wkvp, \
                 tc.tile_pool(name="xqs", bufs=2) as xqs, \
                 tc.tile_pool(name="kevs", bufs=8) as kevs, \
                 tc.tile_pool(name="ktabs", bufs=2) as ktabs, \
                 tc.tile_pool(name="krtmp", bufs=2) as krtmp:
                wk16 = wkvp.tile([P, KO, RB], dt16, tag="wk16")
                wv16 = wkvp.tile([P, KO, RB], dt16, tag="wv16")
                for kc in range(4):
                    nc.sync.dma_start(
                        wk16[:, kc * 8:(kc + 1) * 8, :],
                        wk_c[kc * 8 * P:(kc + 1) * 8 * P, :].rearrange(
                            "(ko p) m -> p ko m", p=P))
                    nc.sync.dma_start(
                        wv16[:, kc * 8:(kc + 1) * 8, :],
                        wv_c[kc * 8 * P:(kc + 1) * 8 * P, :].rearrange(
                            "(ko p) m -> p ko m", p=P))

                # merged K+V over GLOBAL 512-col/row groups: group g covers
                # global blocks 4g..4g+3, assembled from 4 gathered chunks;
                # kT column-pairs complete in order -> gathers pipeline
                for g in range(NCH):
                    jj, n0 = g // 2, (4 * g) % 8
                    xqg = xqs.tile([P, KO, CH], dt16, tag="xqg")
                    for q in range(4):
                        nc.sync.dma_start(
                            xqg[:, :, q * P:(q + 1) * P],
                            xqa[n0 + q, :, :, jj * P:(jj + 1) * P])
                    # K chains for this column group
                    kev = [kevs.tile([P, CH], dt16, tag="kev", name=f"kev{i}")
                           for i in range(4)]
                    for m4 in range(4):
                        ps = psQ.tile([P, CH], dt32, tag="mm")
                        for k in range(KO):
                            nc.tensor.matmul(
                                ps, lhsT=wk16[:, k, m4 * P:(m4 + 1) * P],
                                rhs=xqg[:, k, :],
                                start=(k == 0), stop=(k == KO - 1))
                        nc.scalar.copy(kev[m4], ps)
                    for h in range(2):
                        ck = ktabs.tile([P, CH], dt16, tag="ck")
                        nc.sync.dma_start(
                            ck, cosk_d[h * P:(h + 1) * P, g * CH:(g + 1) * CH])
                        sk = ktabs.tile([P, CH], dt16, tag="sk")
                        nc.sync.dma_start(
                            sk, sink_d[h * P:(h + 1) * P, g * CH:(g + 1) * CH])
                        t1 = krtmp.tile([P, CH], dt16, tag="t1")
                        nc.vector.tensor_tensor(t1, kev[h], ck, mult)
                        t3 = krtmp.tile([P, CH], dt16, tag="t3")
                        nc.vector.tensor_tensor(t3, kev[h], sk, mult)
                        t4 = krtmp.tile([P, CH], dt16, tag="t4")
                        nc.vector.tensor_tensor(t4, kev[2 + h], sk, mult)
                        kroE = kevs.tile([P, CH], dt16, tag="kro", name="kroE")
                        nc.vector.tensor_tensor(kroE, t1, t4, addop)
                        t5 = krtmp.tile([P, CH], dt16, tag="t5")
                        nc.vector.tensor_tensor(t5, kev[2 + h], ck, mult)
                        kroO = kevs.tile([P, CH], dt16, tag="kro", name="kroO")
                        nc.vector.tensor_tensor(kroO, t5, t3, subop)
                        lc = (g % 2) * CH  # col offset within the pair buffer
                        nc.sync.dma_start(
                            kT_b[g // 2][h * P:(h + 1) * P, lc:lc + CH], kroE)
                        nc.sync.dma_start(
                            kT_b[g // 2][(2 + h) * P:(3 + h) * P, lc:lc + CH],
                            kroO)
                    if g % 2 == 1:  # column pair complete -> gather it now
                        nc.gpsimd.collective_compute(
                            "AllGather", mybir.AluOpType.bypass,
                            replica_groups=RG, ins=[kT_b[g // 2][:]],
                            outs=[kT_all[g // 2][:]])
                    # V chains for the same global row group (xqg reused)
                    vdst = v_b0 if g < 4 else v_b1
                    for ms in range(4):
                        ps = psQ.tile([P, CH], dt32, tag="mm")
                        for k in range(KO):
                            nc.tensor.matmul(
                                ps, lhsT=xqg[:, k, ms * P:(ms + 1) * P],
                                rhs=wv16[:, k, :],
                                start=(k == 0), stop=(k == KO - 1))
                        vev = kevs.tile([P, CH], dt16, tag="vev")
                        nc.scalar.copy(vev, ps)
                        gb = (4 * (g % 4) + ms) * P  # row offset in the half
                        nc.sync.dma_start(vdst[gb:gb + P, :], vev)
                    if g == 3:
                        nc.gpsimd.collective_compute(
                            "AllGather", mybir.AluOpType.bypass,
                            replica_groups=RG, ins=[v_b0[:]], outs=[v_all0[:]])
                nc.gpsimd.collective_compute(
                    "AllGather", mybir.AluOpType.bypass, replica_groups=RG,
                    ins=[v_b1[:]], outs=[v_all1[:]])

            with tc.tile_pool(name="PTp", bufs=1) as PTp:
                PT = PTp.tile([P, KO, RB], dt16, tag="PT")

                # ---- scores + softmax + P^T ----
                kta = [kT_all[p][:].rearrange("(ko p) s -> p ko s", p=P)
                       for p in range(4)]
                with tc.tile_pool(name="qTtp", bufs=1) as qTtp, \
                     tc.tile_pool(name="Pp", bufs=1) as Pp, \
                     tc.tile_pool(name="kts", bufs=2) as kts, \
                     tc.tile_pool(name="sstat", bufs=4) as sstat, \
                     tc.tile_pool(name="ams", bufs=2) as ams:
                    qTt = qTtp.tile([P, KO, RB], dt16, tag="qTt")
                    nc.sync.dma_start(
                        qTt, qT_d[:].rearrange("(ko p) s -> p ko s", p=P))
                    Ptile = Pp.tile([P, 4, S], dt16, tag="P")

                    def softmax(m):
                        L = (2 * m + 2) * CH  # causal prefix length
                        maxv = sstat.tile([P, 1], dt32, tag="mx")
                        nc.vector.tensor_reduce(
                            maxv, Ptile[:, m, :L], axis=AX, op=maxop)
                        negb = sstat.tile([P, 1], dt32, tag="nb")
                        nc.vector.tensor_scalar(
                            negb, maxv, -1.0 / 64.0, -LOG64, mult, addop)
                        ssum = sstat.tile([P, 1], dt32, tag="sm")
                        nc.scalar.activation(
                            Ptile[:, m, :L], Ptile[:, m, :L], Exp,
                            bias=negb[:, 0:1], scale=1.0 / 64.0,
                            accum_out=ssum)
                        nc.vector.reciprocal(rinv_all[:, m:m + 1], ssum)

                    def p_transpose(m):
                        for st in range(8 * (m + 1)):
                            pt = pstr.tile([P, P], dt16, tag="tr")
                            nc.tensor.transpose(
                                pt, Ptile[:, m, st * P:(st + 1) * P], ident)
                            nc.scalar.copy(PT[:, st, m * P:(m + 1) * P], pt)

                    for n in range(NCH):
                        allowed = [m4 for m4 in range(4) if n <= 2 * m4 + 1]
                        kt = kts.tile([P, KO, CH], dt16, tag="kt")
                        lc = (n % 2) * CH
                        nc.sync.dma_start(kt, kta[n // 2][:, :, lc:lc + CH])
                        for m4 in allowed:
                            ps = psQ.tile([P, CH], dt32, tag="mm")
                            for k in range(KO):
                                nc.tensor.matmul(
                                    ps, lhsT=qTt[:, k, m4 * P:(m4 + 1) * P],
                                    rhs=kt[:, k, :],
                                    start=(k == 0), stop=(k == KO - 1))
                            if n in (2 * m4, 2 * m4 + 1):
                                am = ams.tile([P, CH], dt16, tag="am")
                                nc.sync.dma_start(
                                    am, amask[m4 * P:(m4 + 1) * P,
                                              (n - 2 * m4) * CH:
                                              (n - 2 * m4 + 1) * CH])
                                nc.vector.tensor_tensor(
                                    Ptile[:, m4, n * CH:(n + 1) * CH],
                                    ps, am, addop)
                            else:
                                nc.scalar.copy(
                                    Ptile[:, m4, n * CH:(n + 1) * CH], ps)
                        if n == 1:
                            softmax(0)
                            p_transpose(0)
                        elif n == 3:
                            softmax(1)
                            p_transpose(1)
                        elif n == 5:
                            softmax(2)
                        elif n == 6:
                            p_transpose(2)
                        elif n == 7:
                            softmax(3)
                            p_transpose(3)

                with tc.tile_pool(name="attnTp", bufs=1) as attnTp:
                    attnT = attnTp.tile([P, KO, RB], dt16, tag="attnT")

                    # v halves are in GLOBAL row order: va0 = key blocks 0..15
                    va0 = v_all0[:].rearrange(
                        "(c l p) eo -> c p l eo", c=NC, p=P)
                    va1 = v_all1[:].rearrange(
                        "(c l p) eo -> c p l eo", c=NC, p=P)

                    with tc.tile_pool(name="wos", bufs=2) as wos, \
                         tc.tile_pool(name="vs", bufs=4) as vs, \
                         tc.tile_pool(name="oev", bufs=4) as oev:
                        # ---- attn^T = V x P^T ----
                        for m in range(KO):
                            c, sub = m // 4, m % 4
                            vlo = vs.tile([P, KO // 2, P], dt16, tag="vlo")
                            nc.sync.dma_start(
                                vlo, va0[c, :, :, sub * P:(sub + 1) * P])
                            vhi = vs.tile([P, KO // 2, P], dt16, tag="vhi")
                            nc.sync.dma_start(
                                vhi, va1[c, :, :, sub * P:(sub + 1) * P])
                            ps = psQ.tile([P, CH], dt32, tag="mm")
                            for k in range(KO):
                                vt = vlo[:, k, :] if k < 16 else vhi[:, k - 16, :]
                                j0 = k // 8
                                nc.tensor.matmul(
                                    ps[:, j0 * P:], lhsT=vt,
                                    rhs=PT[:, k, j0 * P:],
                                    start=(k == 0), stop=(k == KO - 1))
                            nc.scalar.copy(attnT[:, m, :], ps)

                        # ---- out = attn @ wo, rows stay ours; /sum here ----
                        for n in range(NCH):
                            wot = wos.tile([P, KO, CH], dt16, tag="wot")
                            nc.sync.dma_start(
                                wot, wo[:, n * CH:(n + 1) * CH].rearrange(
                                    "(ko p) m -> p ko m", p=P))
                            for mq in range(4):
                                ps = psQ.tile([P, CH], dt32, tag="mm")
                                for k in range(KO):
                                    nc.tensor.matmul(
                                        ps,
                                        lhsT=attnT[:, k, mq * P:(mq + 1) * P],
                                        rhs=wot[:, k, :],
                                        start=(k == 0), stop=(k == KO - 1))
                                ot = oev.tile([P, CH], dt16, tag="ot")
                                nc.scalar.activation(
                                    ot, ps, Copy,
                                    scale=rinv_all[:, mq:mq + 1])
                                nc.sync.dma_start(
                                    out_r[mq * P:(mq + 1) * P,
                                          n * CH:(n + 1) * CH], ot)

    nc.compile()
    return nc


def _feature_perm():
    # per 512-block: first 256 even global features, then 256 odd
    blocks = []
    for c in range(NC):
        j = np.arange(256) + c * 256
        blocks.append(2 * j)
        blocks.append(2 * j + 1)
    return np.concatenate(blocks)


def _rope_tables():
    # feature-major half tables: theta[j, s] = s / BASE^(2j/E), j<E/2
    j = np.arange(HALF, dtype=np.float64)[:, None]
    pos = np.arange(S, dtype=np.float64)[None, :]
    theta = pos / np.power(np.float64(BASE_THETA), 2.0 * j / np.float64(E))
    return (np.cos(theta).astype(np.float16),
            np.sin(theta).astype(np.float16))


def _own_rows(r):
    # core r owns 128-row blocks {8j + r : j=0..3}
    return np.concatenate(
        [np.arange(128 * (8 * j + r), 128 * (8 * j + r) + 128) for j in range(4)])


def _prep_in_maps(inputs):
    f16 = np.float16
    x = np.ascontiguousarray(np.asarray(inputs["x"], dtype=np.float32))
    sf = np.asarray(inputs["scaling_factor"], dtype=np.float32)[:, None]
    wq_s = (sf * np.asarray(inputs["w_q"], dtype=np.float32)).astype(f16)
    wk_s = (sf * np.asarray(inputs["w_k"], dtype=np.float32)).astype(f16)
    wv_s = (sf * np.asarray(inputs["w_v"], dtype=np.float32)).astype(f16)
    wo_s = np.asarray(inputs["w_out"], dtype=np.float32).astype(f16)

    perm = _feature_perm()
    wq_p = np.ascontiguousarray(wq_s[:, perm])
    wk_p = np.ascontiguousarray(wk_s[:, perm])
    cosT, sinT = _rope_tables()

    col = np.arange(S)[None, :]
    in_maps = []
    for r in range(NC):
        rows = _own_rows(r)
        am = np.zeros((RB, 2 * CH), dtype=f16)
        for m4 in range(4):
            rblk = rows[m4 * P:(m4 + 1) * P][:, None]
            cols = np.arange(2 * m4 * CH, (2 * m4 + 2) * CH)[None, :]
            am[m4 * P:(m4 + 1) * P] = np.where(
                cols > rblk, f16(-np.inf), f16(0.0))
        jsh = np.arange(r * 256, (r + 1) * 256)  # this core's rope rows (K)
        in_maps.append({
            "x_r": np.ascontiguousarray(x[rows, :]),
            "wq": wq_p,
            "wk_c": np.ascontiguousarray(wk_p[:, r * RB:(r + 1) * RB]),
            "wv_c": np.ascontiguousarray(wv_s[:, r * RB:(r + 1) * RB]),
            "wo": wo_s,
            "cosq": np.ascontiguousarray(cosT[:, rows]),
            "sinq": np.ascontiguousarray(sinT[:, rows]),
            "cosk": np.ascontiguousarray(cosT[jsh]),
            "sink": np.ascontiguousarray(sinT[jsh]),
            "amask": am,
        })
    return in_maps


def _run(inputs, trace=False, **kw):
    global _BUILT
    from concourse.bass_utils import run_bass_kernel_spmd
    if _BUILT is None:
        _BUILT = _build_nc()
    in_maps = _prep_in_maps(inputs)
    res = run_bass_kernel_spmd(_BUILT, in_maps, list(range(NC)), trace=trace, **kw)
    out = np.empty((S, E), dtype=np.float16)
    for r in range(NC):
        out[_own_rows(r)] = np.asarray(res.results[r]["out_r"]).astype(np.float16)
    return out, res


def kernel(**inputs):
    out, _ = _run(inputs, trace=False)
    return out


# revision 26
# speedup vs baseline: 1.0181x; 1.0181x over previous
"""Bass/Tile TP attention kernel for trn2, 8 NeuronCores.

Sequence-parallel attention tuned for a gapless PE stream (~226 ns per
512-wide fp16 matmul incl. hidden FWL weight loads) and for HBM traffic
(the phases are near the compute/memory ridge):

  - weights pre-scaled by scaling_factor and cast to fp16 on host
  - wq/wk columns host-permuted even/odd per 512-block so RoPE pairs sit
    in separate 128-row tiles -> rope is pure element-wise DVE work
  - phases: norm -> [xq AllGather || Q proj (deep wq prefetch)] ->
    merged K/V loop (K frontloaded, V lags 2 chunks; xq read ONCE) ->
    [kT AllGather || V tail] -> [v half-gathers || scores+softmax] ->
    attnV (wo prefetch) -> out proj
  - qT spilled to DRAM between Q and scores to free SBUF for 3-deep xq
    chunk buffering in the merged loop
  - softmax: 1/64 folded into exp scale, ln64 shift keeps unnormalized
    P/attn in fp16 range, 1/sum applied free at out-proj psum eviction
  - v_b kept in chunk order, gathered in 2 halves, k-index remapped in
    attnV so the second half is never on the critical path

core r owns query row blocks {128*(8j+r) : j=0..3} (causal balance).
"""

import numpy as np

S = 4096
E = 4096
NC = 8
RB = S // NC          # 512 rows per core
P = 128
KO = E // P           # 32 k-tiles
NCH = 8               # 512-wide chunks over S or E
CH = 512
HALF = E // 2
EPS = 1e-6
BASE_THETA = 10000.0
LOG64 = float(np.log(64.0))

_BUILT = None


def _build_nc():
    import concourse.bass as bass
    import concourse.mybir as mybir
    import concourse.tile as tile
    from concourse import bacc
    from concourse.masks import make_identity

    dt16 = mybir.dt.float16
    dt32 = mybir.dt.float32
    AX = mybir.AxisListType.X
    mult = mybir.AluOpType.mult
    addop = mybir.AluOpType.add
    subop = mybir.AluOpType.subtract
    maxop = mybir.AluOpType.max
    Copy = mybir.ActivationFunctionType.Copy
    Exp = mybir.ActivationFunctionType.Exp
    Square = mybir.ActivationFunctionType.Square

    nc = bacc.Bacc(
        "TRN2", target_bir_lowering=False, debug=False, num_devices=NC)

    # I/O (weights arrive fp16, pre-scaled, wq/wk col-permuted even/odd)
    x_r = nc.dram_tensor("x_r", [RB, E], dt32, kind="ExternalInput")
    wq = nc.dram_tensor("wq", [E, E], dt16, kind="ExternalInput")
    wk_c = nc.dram_tensor("wk_c", [E, RB], dt16, kind="ExternalInput")
    wv_c = nc.dram_tensor("wv_c", [E, RB], dt16, kind="ExternalInput")
    wo = nc.dram_tensor("wo", [E, E], dt16, kind="ExternalInput")
    cosq_d = nc.dram_tensor("cosq", [HALF, RB], dt16, kind="ExternalInput")
    sinq_d = nc.dram_tensor("sinq", [HALF, RB], dt16, kind="ExternalInput")
    cosk_d = nc.dram_tensor("cosk", [2 * P, S], dt16, kind="ExternalInput")
    sink_d = nc.dram_tensor("sink", [2 * P, S], dt16, kind="ExternalInput")
    amask = nc.dram_tensor("amask", [RB, 2 * CH], dt16, kind="ExternalInput")
    out_r = nc.dram_tensor("out_r", [RB, E], dt16, kind="ExternalOutput")

    # internal DRAM
    qT_d = nc.dram_tensor("qT_d", [E, RB], dt16)  # roped q^T spill
    xqT_b = nc.dram_tensor("xqT_b", [E, RB], dt16)
    xqT_all = nc.dram_tensor("xqT_all", [NC * E, RB], dt16, addr_space="Shared")
    # kT in 4 column-pair buffers so gathers pipeline under the KV loop
    kT_b = [nc.dram_tensor(f"kT_b{p}", [RB, 2 * CH], dt16) for p in range(4)]
    kT_all = [nc.dram_tensor(f"kT_all{p}", [E, 2 * CH], dt16,
                             addr_space="Shared") for p in range(4)]
    v_b0 = nc.dram_tensor("v_b0", [S // 2, RB], dt16)
    v_b1 = nc.dram_tensor("v_b1", [S // 2, RB], dt16)
    v_all0 = nc.dram_tensor("v_all0", [NC * S // 2, RB], dt16, addr_space="Shared")
    v_all1 = nc.dram_tensor("v_all1", [NC * S // 2, RB], dt16, addr_space="Shared")
    warm_b = nc.dram_tensor("warm_b", [P, 8], dt16)
    warm_all = nc.dram_tensor("warm_all", [NC * P, 8], dt16, addr_space="Shared")
    RG = [list(range(NC))]

    with tile.TileContext(nc) as tc:
        with tc.tile_pool(name="const", bufs=1) as constp, \
             tc.tile_pool(name="psQ", bufs=4, space="PSUM") as psQ, \
             tc.tile_pool(name="pstr", bufs=4, space="PSUM") as pstr:

            ident = constp.tile([P, P], dt16, tag="ident")
            make_identity(nc, ident)
            rinv_all = constp.tile([P, 4], dt32, tag="rinv_all")

            # tiny warmup gather: absorbs the first-collective rendezvous
            # barrier (core-start skew) under the norm/Q phases
            warm = constp.tile([P, 8], dt16, tag="warm")
            nc.vector.memset(warm, 0.0)
            nc.sync.dma_start(warm_b[:], warm)
            nc.gpsimd.collective_compute(
                "AllGather", mybir.AluOpType.bypass, replica_groups=RG,
                ins=[warm_b[:]], outs=[warm_all[:]])

            with tc.tile_pool(name="xqTrp", bufs=1) as xqTrp:
                xqT_r = xqTrp.tile([P, KO, RB], dt16, tag="xqT_r")

                # ---- stage A: RMS norm of own rows + transpose ----
                with tc.tile_pool(name="normp", bufs=2) as normp, \
                     tc.tile_pool(name="nstat", bufs=2) as nstat:
                    for t in range(RB // P):
                        x_sb = normp.tile([P, E], dt32, tag="x")
                        nc.sync.dma_start(x_sb, x_r[t * P:(t + 1) * P, :])
                        sq = normp.tile([P, E], dt32, tag="sq")
                        ssum = nstat.tile([P, 1], dt32, tag="ssum")
                        nc.scalar.activation(sq, x_sb, Square, accum_out=ssum)
                        s2 = nstat.tile([P, 1], dt32, tag="s2")
                        nc.vector.tensor_scalar(s2, ssum, 1.0 / E, EPS, mult, addop)
                        s3 = nstat.tile([P, 1], dt32, tag="s3")
                        nc.scalar.sqrt(s3, s2)
                        rinv = nstat.tile([P, 1], dt32, tag="rinv")
                        nc.vector.reciprocal(rinv, s3)
                        xq_sb = normp.tile([P, E], dt16, tag="xq")
                        nc.vector.tensor_scalar_mul(xq_sb, x_sb, rinv[:, 0:1])
                        for c in range(KO):
                            pt = pstr.tile([P, P], dt16, tag="tr")
                            nc.tensor.transpose(pt, xq_sb[:, c * P:(c + 1) * P], ident)
                            nc.scalar.copy(xqT_r[:, c, t * P:(t + 1) * P], pt)
                        nc.sync.dma_start(
                            xqT_b[:, t * P:(t + 1) * P].rearrange(
                                "(ko p) s -> p ko s", p=P),
                            xqT_r[:, :, t * P:(t + 1) * P])

                nc.gpsimd.collective_compute(
                    "AllGather", mybir.AluOpType.bypass, replica_groups=RG,
                    ins=[xqT_b[:]], outs=[xqT_all[:]])

                # ---- stage QT: qT = wq^T @ xq^T for own rows, + rope ----
                # deep wq prefetch (4 groups = 16MB) rides ahead of the
                # gather's HBM traffic; roped q^T spills to DRAM
                with tc.tile_pool(name="wqs", bufs=4) as wqs, \
                     tc.tile_pool(name="qring", bufs=8) as qring, \
                     tc.tile_pool(name="qropes", bufs=2) as qropes, \
                     tc.tile_pool(name="qrtmp", bufs=2) as qrtmp:
                    for mg in range(8):
                        wqt = wqs.tile([P, KO, CH], dt16, tag="wqt")
                        for hh in range(2):  # split across DMA engines
                            nc.sync.dma_start(
                                wqt[:, hh * 16:(hh + 1) * 16, :],
                                wq[hh * 16 * P:(hh + 1) * 16 * P,
                                   mg * CH:(mg + 1) * CH].rearrange(
                                    "(ko p) m -> p ko m", p=P))
                        qg = [qring.tile([P, RB], dt16, tag="qg", name=f"qg{i}")
                              for i in range(4)]
                        for m4 in range(4):
                            ps = psQ.tile([P, CH], dt32, tag="mm")
                            for k in range(KO):
                                nc.tensor.matmul(
                                    ps, lhsT=wqt[:, k, m4 * P:(m4 + 1) * P],
                                    rhs=xqT_r[:, k, :],
                                    start=(k == 0), stop=(k == KO - 1))
                            nc.scalar.copy(qg[m4], ps)
                        # rope pairs (h, 2+h) within this 512-col block
                        for h in range(2):
                            j0 = mg * 2 + h  # 128-row block into cosq/sinq
                            cq = qropes.tile([P, RB], dt16, tag="cq")
                            nc.sync.dma_start(cq, cosq_d[j0 * P:(j0 + 1) * P, :])
                            sq_ = qropes.tile([P, RB], dt16, tag="sq")
                            nc.sync.dma_start(sq_, sinq_d[j0 * P:(j0 + 1) * P, :])
                            t1 = qrtmp.tile([P, RB], dt16, tag="t1")
                            nc.vector.tensor_tensor(t1, qg[h], cq, mult)
                            t3 = qrtmp.tile([P, RB], dt16, tag="t3")
                            nc.vector.tensor_tensor(t3, qg[h], sq_, mult)
                            t4 = qrtmp.tile([P, RB], dt16, tag="t4")
                            nc.vector.tensor_tensor(t4, qg[2 + h], sq_, mult)
                            qE = qring.tile([P, RB], dt16, tag="qro", name="qE")
                            nc.vector.tensor_tensor(qE, t1, t4, addop)
                            t5 = qrtmp.tile([P, RB], dt16, tag="t5")
                            nc.vector.tensor_tensor(t5, qg[2 + h], cq, mult)
                            qO = qring.tile([P, RB], dt16, tag="qro", name="qO")
                            nc.vector.tensor_tensor(qO, t5, t3, subop)
                            me, mo = mg * 4 + h, mg * 4 + 2 + h
                            nc.sync.dma_start(qT_d[me * P:(me + 1) * P, :], qE)
                            nc.sync.dma_start(qT_d[mo * P:(mo + 1) * P, :], qO)

            # xqT_r freed
            xqa = xqT_all[:].rearrange("(c ko p) s -> c p ko s", c=NC, p=P)
            with tc.tile_pool(name="wkvp", bufs=1) as wkvp, \
                 tc.tile_pool(name="xqs", bufs=3) as xqs, \
                 tc.tile_pool(name="kevs", bufs=8) as kevs, \
                 tc.tile_pool(name="ktabs", bufs=2) as ktabs, \
                 tc.tile_pool(name="krtmp", bufs=2) as krtmp:
                wk16 = wkvp.tile([P, KO, RB], dt16, tag="wk16")
                wv16 = wkvp.tile([P, KO, RB], dt16, tag="wv16")
                for kc in range(4):
                    nc.sync.dma_start(
                        wk16[:, kc * 8:(kc + 1) * 8, :],
                        wk_c[kc * 8 * P:(kc + 1) * 8 * P, :].rearrange(
                            "(ko p) m -> p ko m", p=P))
                    nc.sync.dma_start(
                        wv16[:, kc * 8:(kc + 1) * 8, :],
                        wv_c[kc * 8 * P:(kc + 1) * 8 * P, :].rearrange(
                            "(ko p) m -> p ko m", p=P))

                # merged K+V over GLOBAL 512-col/row groups: group g covers
                # global blocks 4g..4g+3, assembled from 4 gathered chunks;
                # kT column-pairs complete in order -> gathers pipeline
                def assemble(g):
                    jj, n0 = g // 2, (4 * g) % 8
                    xqg = xqs.tile([P, KO, CH], dt16, tag="xqg")
                    for q in range(4):
                        nc.sync.dma_start(
                            xqg[:, :, q * P:(q + 1) * P],
                            xqa[n0 + q, :, :, jj * P:(jj + 1) * P])
                    return xqg

                xq_pipe = [assemble(0), assemble(1)]
                for g in range(NCH):
                    xqg = xq_pipe.pop(0)
                    if g + 2 < NCH:
                        xq_pipe.append(assemble(g + 2))
                    # K chains for this column group
                    kev = [kevs.tile([P, CH], dt16, tag="kev", name=f"kev{i}")
                           for i in range(4)]
                    for m4 in range(4):
                        ps = psQ.tile([P, CH], dt32, tag="mm")
                        for k in range(KO):
                            nc.tensor.matmul(
                                ps, lhsT=wk16[:, k, m4 * P:(m4 + 1) * P],
                                rhs=xqg[:, k, :],
                                start=(k == 0), stop=(k == KO - 1))
                        nc.scalar.copy(kev[m4], ps)
                    for h in range(2):
                        ck = ktabs.tile([P, CH], dt16, tag="ck")
                        nc.sync.dma_start(
                            ck, cosk_d[h * P:(h + 1) * P, g * CH:(g + 1) * CH])
                        sk = ktabs.tile([P, CH], dt16, tag="sk")
                        nc.sync.dma_start(
                            sk, sink_d[h * P:(h + 1) * P, g * CH:(g + 1) * CH])
                        t1 = krtmp.tile([P, CH], dt16, tag="t1")
                        nc.vector.tensor_tensor(t1, kev[h], ck, mult)
                        t3 = krtmp.tile([P, CH], dt16, tag="t3")
                        nc.vector.tensor_tensor(t3, kev[h], sk, mult)
                        t4 = krtmp.tile([P, CH], dt16, tag="t4")
                        nc.vector.tensor_tensor(t4, kev[2 + h], sk, mult)
                        kroE = kevs.tile([P, CH], dt16, tag="kro", name="kroE")
                        nc.vector.tensor_tensor(kroE, t1, t4, addop)
                        t5 = krtmp.tile([P, CH], dt16, tag="t5")
                        nc.vector.tensor_tensor(t5, kev[2 + h], ck, mult)
                        kroO = kevs.tile([P, CH], dt16, tag="kro", name="kroO")
                        nc.vector.tensor_tensor(kroO, t5, t3, subop)
                        lc = (g % 2) * CH  # col offset within the pair buffer
                        nc.sync.dma_start(
                            kT_b[g // 2][h * P:(h + 1) * P, lc:lc + CH], kroE)
                        nc.sync.dma_start(
                            kT_b[g // 2][(2 + h) * P:(3 + h) * P, lc:lc + CH],
                            kroO)
                    if g % 2 == 1:  # column pair complete -> gather it now
                        nc.gpsimd.collective_compute(
                            "AllGather", mybir.AluOpType.bypass,
                            replica_groups=RG, ins=[kT_b[g // 2][:]],
                            outs=[kT_all[g // 2][:]])
                    # V chains for the same global row group (xqg reused)
                    vdst = v_b0 if g < 4 else v_b1
                    for ms in range(4):
                        ps = psQ.tile([P, CH], dt32, tag="mm")
                        for k in range(KO):
                            nc.tensor.matmul(
                                ps, lhsT=xqg[:, k, ms * P:(ms + 1) * P],
                                rhs=wv16[:, k, :],
                                start=(k == 0), stop=(k == KO - 1))
                        vev = kevs.tile([P, CH], dt16, tag="vev")
                        nc.scalar.copy(vev, ps)
                        gb = (4 * (g % 4) + ms) * P  # row offset in the half
                        nc.sync.dma_start(vdst[gb:gb + P, :], vev)
                    if g == 3:
                        nc.gpsimd.collective_compute(
                            "AllGather", mybir.AluOpType.bypass,
                            replica_groups=RG, ins=[v_b0[:]], outs=[v_all0[:]])
                nc.gpsimd.collective_compute(
                    "AllGather", mybir.AluOpType.bypass, replica_groups=RG,
                    ins=[v_b1[:]], outs=[v_all1[:]])

            with tc.tile_pool(name="PTp", bufs=1) as PTp:
                PT = PTp.tile([P, KO, RB], dt16, tag="PT")

                # ---- scores + softmax + P^T ----
                kta = [kT_all[p][:].rearrange("(ko p) s -> p ko s", p=P)
                       for p in range(4)]
                with tc.tile_pool(name="qTtp", bufs=1) as qTtp, \
                     tc.tile_pool(name="Pp", bufs=1) as Pp, \
                     tc.tile_pool(name="kts", bufs=2) as kts, \
                     tc.tile_pool(name="sstat", bufs=4) as sstat, \
                     tc.tile_pool(name="ams", bufs=2) as ams:
                    qTt = qTtp.tile([P, KO, RB], dt16, tag="qTt")
                    for hh in range(4):
                        nc.sync.dma_start(
                            qTt[:, hh * 8:(hh + 1) * 8, :],
                            qT_d[hh * 8 * P:(hh + 1) * 8 * P, :].rearrange(
                                "(ko p) s -> p ko s", p=P))
                    Ptile = Pp.tile([P, 4, S], dt16, tag="P")

                    def softmax(m):
                        L = (2 * m + 2) * CH  # causal prefix length
                        maxv = sstat.tile([P, 1], dt32, tag="mx")
                        nc.vector.tensor_reduce(
                            maxv, Ptile[:, m, :L], axis=AX, op=maxop)
                        negb = sstat.tile([P, 1], dt32, tag="nb")
                        nc.vector.tensor_scalar(
                            negb, maxv, -1.0 / 64.0, -LOG64, mult, addop)
                        ssum = sstat.tile([P, 1], dt32, tag="sm")
                        nc.scalar.activation(
                            Ptile[:, m, :L], Ptile[:, m, :L], Exp,
                            bias=negb[:, 0:1], scale=1.0 / 64.0,
                            accum_out=ssum)
                        nc.vector.reciprocal(rinv_all[:, m:m + 1], ssum)

                    def p_transpose(m):
                        for st in range(8 * (m + 1)):
                            pt = pstr.tile([P, P], dt16, tag="tr")
                            nc.tensor.transpose(
                                pt, Ptile[:, m, st * P:(st + 1) * P], ident)
                            nc.scalar.copy(PT[:, st, m * P:(m + 1) * P], pt)

                    for n in range(NCH):
                        allowed = [m4 for m4 in range(4) if n <= 2 * m4 + 1]
                        kt = kts.tile([P, KO, CH], dt16, tag="kt")
                        lc = (n % 2) * CH
                        for hh in range(2):
                            nc.sync.dma_start(
                                kt[:, hh * 16:(hh + 1) * 16, :],
                                kta[n // 2][:, hh * 16:(hh + 1) * 16,
                                            lc:lc + CH])
                        for m4 in allowed:
                            ps = psQ.tile([P, CH], dt32, tag="mm")
                            for k in range(KO):
                                nc.tensor.matmul(
                                    ps, lhsT=qTt[:, k, m4 * P:(m4 + 1) * P],
                                    rhs=kt[:, k, :],
                                    start=(k == 0), stop=(k == KO - 1))
                            if n in (2 * m4, 2 * m4 + 1):
                                am = ams.tile([P, CH], dt16, tag="am")
                                nc.sync.dma_start(
                                    am, amask[m4 * P:(m4 + 1) * P,
                                              (n - 2 * m4) * CH:
                                              (n - 2 * m4 + 1) * CH])
                                nc.vector.tensor_tensor(
                                    Ptile[:, m4, n * CH:(n + 1) * CH],
                                    ps, am, addop)
                            else:
                                nc.scalar.copy(
                                    Ptile[:, m4, n * CH:(n + 1) * CH], ps)
                        if n == 1:
                            softmax(0)
                            p_transpose(0)
                        elif n == 3:
                            softmax(1)
                            p_transpose(1)
                        elif n == 5:
                            softmax(2)
                        elif n == 6:
                            p_transpose(2)
                        elif n == 7:
                            softmax(3)
                            p_transpose(3)

                with tc.tile_pool(name="attnTp", bufs=1) as attnTp:
                    attnT = attnTp.tile([P, KO, RB], dt16, tag="attnT")

                    # v halves are in GLOBAL row order: va0 = key blocks 0..15
                    va0 = v_all0[:].rearrange(
                        "(c l p) eo -> c p l eo", c=NC, p=P)
                    va1 = v_all1[:].rearrange(
                        "(c l p) eo -> c p l eo", c=NC, p=P)

                    with tc.tile_pool(name="wos", bufs=3) as wos, \
                         tc.tile_pool(name="vs", bufs=4) as vs, \
                         tc.tile_pool(name="oev", bufs=4) as oev:
                        # ---- attn^T = V x P^T ----
                        for m in range(KO):
                            c, sub = m // 4, m % 4
                            vlo = vs.tile([P, KO // 2, P], dt16, tag="vlo")
                            nc.sync.dma_start(
                                vlo, va0[c, :, :, sub * P:(sub + 1) * P])
                            vhi = vs.tile([P, KO // 2, P], dt16, tag="vhi")
                            nc.sync.dma_start(
                                vhi, va1[c, :, :, sub * P:(sub + 1) * P])
                            ps = psQ.tile([P, CH], dt32, tag="mm")
                            for k in range(KO):
                                vt = vlo[:, k, :] if k < 16 else vhi[:, k - 16, :]
                                j0 = k // 8
                                nc.tensor.matmul(
                                    ps[:, j0 * P:], lhsT=vt,
                                    rhs=PT[:, k, j0 * P:],
                                    start=(k == 0), stop=(k == KO - 1))
                            nc.scalar.copy(attnT[:, m, :], ps)

                        # ---- out = attn @ wo, rows stay ours; /sum here ----
                        for n in range(NCH):
                            wot = wos.tile([P, KO, CH], dt16, tag="wot")
                            for hh in range(2):
                                nc.sync.dma_start(
                                    wot[:, hh * 16:(hh + 1) * 16, :],
                                    wo[hh * 16 * P:(hh + 1) * 16 * P,
                                       n * CH:(n + 1) * CH].rearrange(
                                        "(ko p) m -> p ko m", p=P))
                            for mq in range(4):
                                ps = psQ.tile([P, CH], dt32, tag="mm")
                                for k in range(KO):
                                    nc.tensor.matmul(
                                        ps,
                                        lhsT=attnT[:, k, mq * P:(mq + 1) * P],
                                        rhs=wot[:, k, :],
                                        start=(k == 0), stop=(k == KO - 1))
                                ot = oev.tile([P, CH], dt16, tag="ot")
                                nc.scalar.activation(
                                    ot, ps, Copy,
                                    scale=rinv_all[:, mq:mq + 1])
                                nc.sync.dma_start(
                                    out_r[mq * P:(mq + 1) * P,
                                          n * CH:(n + 1) * CH], ot)

    nc.compile()
    return nc


def _feature_perm():
    # per 512-block: first 256 even global features, then 256 odd
    blocks = []
    for c in range(NC):
        j = np.arange(256) + c * 256
        blocks.append(2 * j)
        blocks.append(2 * j + 1)
    return np.concatenate(blocks)


def _rope_tables():
    # feature-major half tables: theta[j, s] = s / BASE^(2j/E), j<E/2
    j = np.arange(HALF, dtype=np.float64)[:, None]
    pos = np.arange(S, dtype=np.float64)[None, :]
    theta = pos / np.power(np.float64(BASE_THETA), 2.0 * j / np.float64(E))
    return (np.cos(theta).astype(np.float16),
            np.sin(theta).astype(np.float16))


def _own_rows(r):
    # core r owns 128-row blocks {8j + r : j=0..3}
    return np.concatenate(
        [np.arange(128 * (8 * j + r), 128 * (8 * j + r) + 128) for j in range(4)])


def _prep_in_maps(inputs):
    f16 = np.float16
    x = np.ascontiguousarray(np.asarray(inputs["x"], dtype=np.float32))
    sf = np.asarray(inputs["scaling_factor"], dtype=np.float32)[:, None]
    wq_s = (sf * np.asarray(inputs["w_q"], dtype=np.float32)).astype(f16)
    wk_s = (sf * np.asarray(inputs["w_k"], dtype=np.float32)).astype(f16)
    wv_s = (sf * np.asarray(inputs["w_v"], dtype=np.float32)).astype(f16)
    wo_s = np.asarray(inputs["w_out"], dtype=np.float32).astype(f16)

    perm = _feature_perm()
    wq_p = np.ascontiguousarray(wq_s[:, perm])
    wk_p = np.ascontiguousarray(wk_s[:, perm])
    cosT, sinT = _rope_tables()

    col = np.arange(S)[None, :]
    in_maps = []
    for r in range(NC):
        rows = _own_rows(r)
        am = np.zeros((RB, 2 * CH), dtype=f16)
        for m4 in range(4):
            rblk = rows[m4 * P:(m4 + 1) * P][:, None]
            cols = np.arange(2 * m4 * CH, (2 * m4 + 2) * CH)[None, :]
            am[m4 * P:(m4 + 1) * P] = np.where(
                cols > rblk, f16(-np.inf), f16(0.0))
        jsh = np.arange(r * 256, (r + 1) * 256)  # this core's rope rows (K)
        in_maps.append({
            "x_r": np.ascontiguousarray(x[rows, :]),
            "wq": wq_p,
            "wk_c": np.ascontiguousarray(wk_p[:, r * RB:(r + 1) * RB]),
            "wv_c": np.ascontiguousarray(wv_s[:, r * RB:(r + 1) * RB]),
            "wo": wo_s,
            "cosq": np.ascontiguousarray(cosT[:, rows]),
            "sinq": np.ascontiguousarray(sinT[:, rows]),
            "cosk": np.ascontiguousarray(cosT[jsh]),
            "sink": np.ascontiguousarray(sinT[jsh]),
            "amask": am,
        })
    return in_maps


def _run(inputs, trace=False, **kw):
    global _BUILT
    from concourse.bass_utils import run_bass_kernel_spmd
    if _BUILT is None:
        _BUILT = _build_nc()
    in_maps = _prep_in_maps(inputs)
    res = run_bass_kernel_spmd(_BUILT, in_maps, list(range(NC)), trace=trace, **kw)
    out = np.empty((S, E), dtype=np.float16)
    for r in range(NC):
        out[_own_rows(r)] = np.asarray(res.results[r]["out_r"]).astype(np.float16)
    return out, res


def kernel(**inputs):
    out, _ = _run(inputs, trace=False)
    return out
